# revision 36
# baseline (speedup 1.0000x reference)
"""Multi-head attention block on 8 Trainium2 NeuronCores, data-parallel over batch.

Per core (one batch element, S=1024 seq, E=1024 embed, H=16 heads, D=64),
all matmuls in bf16 (inputs cast host-side), fp32 PSUM accumulation:
  xT: x loaded over both HWDGE queues, PE-transposed 8 k-tiles per PSUM bank,
      one wide eviction per m-tile (alternating DVE/ACT)
  V = xT.T @ Wv (seq-major) into V_aug = [V | ones(64)] per head
  qT/kT = W_pair.T @ xT per head-pair, pipelined as PE filler inside the
          previous pair's attention (generator queue, 2 pulls per score step)
  scoresT[s2,s1] = kT.T @ qT, one m-tile per step, both heads row-tiled in
      one 2-bank PSUM tile (true double buffering of the score ring)
  exp on ACT, one call per step (PSUM->SBUF; no max-subtract: logits are
      ~N(0,1.5) so exp cannot overflow fp32)
  PV: psum[128,512] = V_aug.T @ expT -> rows 0..63 unnorm outT, rows 64..127
      the softmax row-sum replicated 64-wide (free PE broadcast), PV matmuls
      interleaved one step behind the scores
  normalize from PSUM: outT = po[0:64] * (1/po[64:128]) (DVE, no broadcast)
  out = outT.T @ W_out + b_out, overlapped with pair 7: m0/m1 partially
      accumulated (k<=6) inside pair 7 bank 0, m2/m3 + wout-bank-1 m0..m3
      as pair-7 bank-1 filler, m4..m7 after

Weights are de-interleaved host-side: reference W_qkv columns are (h, d, qkv)
with qkv innermost; we feed wqk (pair-blocked [q0q1k0k1...]) and wv ((h,d) order).
"""

import ml_dtypes
import numpy as np

import concourse.bacc as bacc
import concourse.bass as bass
import concourse.mybir as mybir
from concourse.bass_utils import run_bass_kernel_spmd
from concourse.masks import make_identity
from concourse.tile import TileContext
from concourse.tile_rust import add_dep_helper

F32 = mybir.dt.float32
BF16 = mybir.dt.bfloat16
AF = mybir.ActivationFunctionType

S = 1024       # sequence length
E = 1024       # embed dim
H = 16         # heads
D = 64         # head dim
P = 128        # partitions
NP = 8         # head pairs
KT = E // P    # contraction tiles (8)
SM = S // P    # seq tiles of 128 (8)
NB = S // 512  # seq banks of 512 (2)
SCALE = 1.0 / np.sqrt(D)


def build_nc():
    nc = bacc.Bacc(trn_type="TRN2", target_bir_lowering=False)
    x = nc.dram_tensor("x", [S, E], BF16, kind="ExternalInput")
    wqk = nc.dram_tensor("wqk", [E, 2 * E], BF16, kind="ExternalInput")
    wv = nc.dram_tensor("wv", [E, E], BF16, kind="ExternalInput")
    bqk = nc.dram_tensor("bqk", [2 * E], F32, kind="ExternalInput")
    bv = nc.dram_tensor("bv", [E], F32, kind="ExternalInput")
    wout = nc.dram_tensor("wout", [E, E], BF16, kind="ExternalInput")
    bout = nc.dram_tensor("bout", [E], F32, kind="ExternalInput")
    out = nc.dram_tensor("out", [S, E], F32, kind="ExternalOutput")

    with TileContext(nc) as tc:
        with (
            tc.tile_pool(name="const", bufs=1) as constp,
            tc.tile_pool(name="persist", bufs=1) as pers,
            tc.tile_pool(name="psum", bufs=1, space="PSUM") as psp,
        ):
            # ---- constants ----
            ones = constp.tile([1, 512], BF16, tag="ones")
            nc.vector.memset(ones[:], 1.0)

            # ---- persistent arrays ----
            # xTall[:, k, s]: feature-major x, written 8 k-tiles per eviction
            xTall = pers.tile([P, KT, S], BF16, tag="xtall", name="xTall")
            # V_aug: 64 value columns + 64 ones columns per head, so the PV
            # matmul replicates the softmax row-sum across 64 PSUM partitions
            # (free partition-broadcast on the PE; M=128 streams no slower
            # than M=66)
            vaug = [pers.tile([P, H, 2 * D], BF16, tag=f"va{m}", name=f"vaug{m}")
                    for m in range(SM)]
            outT = [pers.tile([P, S], BF16, tag=f"ot{p}", name=f"outT{p}")
                    for p in range(NP)]
            wvall = pers.tile([P, 2, KT, 512], BF16, tag="wvall", name="wvall")

            bvb = constp.tile([P, E], F32, tag="bvb")
            boutb = constp.tile([P, E], F32, tag="boutb")
            with (
                tc.tile_pool(name="ph0", bufs=1) as ph0,
                tc.tile_pool(name="ph2", bufs=1) as ph2,
                tc.tile_pool(name="ph3", bufs=1) as ph3,
            ):
                bvr = ph0.tile([1, E], F32, tag="bvr")
                nc.scalar.dma_start(bvr[:], bv.ap()[None, :])
                botr = ph0.tile([1, E], F32, tag="botr")
                nc.scalar.dma_start(botr[:], bout.ap()[None, :])

                # ---- load x split across both HWDGE queues; PE transposes,
                # 8 k-tiles batched per PSUM bank so DVE evicts each m-tile
                # with ONE wide copy instead of 8 tiny ones.
                # (Concurrent XBAR dma-transposes on the two queues corrupt
                # each other — the XBAR is shared — so transpose on PE.)
                identity = constp.tile([P, P], BF16, tag="ident")
                make_identity(nc, identity)
                # batch DMAs: each dma_start costs ~600ns of sequencer
                # issue time, so load x in 2-tile chunks and wv in quarters
                # (4 issues each instead of 8/16)
                xs2 = []
                for c in range(SM // 2):
                    xst = ph0.tile([P, 2, E], BF16, tag="xs", bufs=4, name="xs")
                    eng = nc.sync if c % 2 == 0 else nc.scalar
                    eng.dma_start(
                        xst[:],
                        x.ap()[c * 2 * P:(c + 1) * 2 * P, :].rearrange(
                            "(m p) e -> p m e", p=P))
                    xs2.append(xst)
                for n in range(2):
                    for kh in range(2):
                        eng = nc.sync if kh == 0 else nc.scalar
                        eng.dma_start(
                            wvall[:, n, kh * 4:(kh + 1) * 4, :],
                            wv.ap()[kh * 512:(kh + 1) * 512,
                                    bass.ts(n, 512)].rearrange(
                                "(k p) c -> p k c", p=P))
                for m in range(SM):
                    tp = psp.tile([P, KT, P], BF16, tag="pv", bufs=2, name="tp")
                    for k in range(KT):
                        nc.tensor.transpose(
                            tp[:, k], xs2[m // 2][:, m % 2, bass.ts(k, P)],
                            identity[:])
                    # alternate evictions between DVE and the idle ACT so
                    # neither paces the transpose stream
                    if m % 2 == 0:
                        nc.vector.tensor_copy(xTall[:, :, bass.ts(m, P)], tp[:])
                    else:
                        nc.scalar.copy(xTall[:, :, bass.ts(m, P)], tp[:])

                # per-partition bias columns for q/k (slow strided DMA; late
                # need, keep behind the transposes on the scalar queue)
                bcols = constp.tile([P, 2 * NP], F32, tag="bcols")
                nc.scalar.dma_start(bcols[:], bqk.ap().rearrange("(f p) -> p f", p=P))

                def load_wq(p):
                    w = ph2.tile([P, KT, 256], BF16, tag="wqk", bufs=2,
                                 name="wqk")
                    nc.sync.dma_start(
                        w[:], wqk.ap()[:, bass.ts(p, 256)].rearrange(
                            "(k q) c -> q k c", q=P))
                    return w

                wq0 = load_wq(0)

                # bias broadcasts in bf16 (fp32 matmuls are 4 cycles/row and
                # would head-block the in-order PE queue for ~7us)
                bvr16 = ph0.tile([1, E], BF16, tag="bvr16")
                nc.vector.tensor_copy(bvr16[:], bvr[:])
                botr16 = ph0.tile([1, E], BF16, tag="botr16")
                nc.vector.tensor_copy(botr16[:], botr[:])
                for n in range(2):
                    cs = bass.ts(n, 512)
                    pb = psp.tile([P, 512], F32, tag="mm", bufs=2, name="pb")
                    nc.tensor.matmul(pb[:], ones[0:1, 0:P], bvr16[0:1, cs])
                    nc.vector.tensor_copy(bvb[:, cs], pb[:])
                    pb2 = psp.tile([P, 512], F32, tag="mm", bufs=2, name="pb2")
                    nc.tensor.matmul(pb2[:], ones[0:1, 0:P], botr16[0:1, cs])
                    nc.vector.tensor_copy(boutb[:, cs], pb2[:])

                # ---- phase 1: V = x @ Wv (+bv), into vaug with ones cols.
                # Transposes interleave with the V groups; V/proj0 PSUM
                # alternates between the idle "mm" and "sc" rings for depth 4
                # so the DVE evictions never gate the PE.
                for m in range(SM):
                    nc.vector.memset(vaug[m][:, :, D:2 * D], 1.0)

                def alt_psum(i, name):
                    if i % 2 == 0:
                        return psp.tile([P, 512], F32, tag="mm", bufs=2,
                                        name=name)
                    t = psp.tile([P, 2, 512], F32, tag="sc", bufs=2, name=name)
                    return t[:, 0]

                def emit_v(n, m):
                    pv = alt_psum(m, "pvps")
                    for k in range(KT):
                        nc.tensor.matmul(
                            pv[:], xTall[:, k, bass.ts(m, P)], wvall[:, n, k],
                            start=(k == 0), stop=(k == KT - 1))
                    nc.vector.tensor_add(
                        vaug[m][:, bass.ts(n, 8), 0:D],
                        pv[:].rearrange("p (h d) -> p h d", h=8),
                        bvb[:, bass.ts(n, 512)].rearrange("p (h d) -> p h d", h=8))

                for n in range(2):
                    for m in range(SM):
                        emit_v(n, m)

                # ---- phase 2: attention, software-pipelined over head pairs ----
                def load_wot(n):
                    w = ph3.tile([P, KT, 512], BF16, tag="wo", bufs=2,
                                 name="wot")
                    nc.sync.dma_start(
                        w[:], wout.ap()[:, bass.ts(n, 512)].rearrange(
                            "(k q) c -> q k c", q=P))
                    return w

                def alloc_qkt():
                    qt = ph2.tile([P, S], BF16, tag="qt", bufs=2, name="qt")
                    kt = ph2.tile([P, S], BF16, tag="kt", bufs=2, name="kt")
                    return qt, kt

                def proj_group(p, wq, qt, kt, which, n, alt=False,
                               act_evict=None):
                    """One 8-matmul projection group, yielding per matmul."""
                    ws = slice(which * P, (which + 1) * P)
                    dst = qt if which == 0 else kt
                    bc = bcols[:, 2 * p + which:2 * p + which + 1]
                    cs = bass.ts(n, 512)
                    if alt:
                        ps = alt_psum(2 * which + n, "pproj")
                    else:
                        ps = psp.tile([P, 512], F32, tag="mm", bufs=2,
                                      name="pproj")
                    for k in range(KT):
                        nc.tensor.matmul(
                            ps[:], wq[:, k, ws], xTall[:, k, cs],
                            start=(k == 0), stop=(k == KT - 1))
                        yield
                    if act_evict is None:
                        act_evict = which == 0 and not alt
                    if act_evict:
                        # q eviction on ACT (~1.4us/bank slack) so the
                        # DVE never gates the mm PSUM ring; Identity
                        # supports the per-partition bias column
                        nc.scalar.activation(
                            dst[:, cs], ps[:], AF.Identity, bias=bc)
                    else:
                        nc.vector.tensor_scalar_add(dst[:, cs], ps[:], bc)

                def proj_mms(p, wq, qt, kt, alt=False, skip_q_n1=False):
                    """Generator yielding after each proj matmul. With
                    skip_q_n1, the q bank-1 group is left out (deferred into
                    the pair's own bank-0 window, which only needs q bank 0
                    plus all of k)."""
                    for which in range(2):  # 0 = q, 1 = k
                        for n in range(NB):
                            if skip_q_n1 and which == 0 and n == 1:
                                continue
                            yield from proj_group(p, wq, qt, kt, which, n,
                                                  alt=alt)

                class FQ:
                    def __init__(self):
                        self.q = []

                    def add(self, g):
                        self.q.append(g)

                    def pull(self, n):
                        while n > 0 and self.q:
                            try:
                                next(self.q[0])
                                n -= 1
                            except StopIteration:
                                self.q.pop(0)

                    def finish(self):
                        self.pull(1 << 30)

                fq = FQ()

                def emit_final_group(n, m, wot, klo=0, khi=KT, pf=None):
                    cs = bass.ts(n, 512)
                    if pf is None:
                        pf = psp.tile([P, 512], F32, tag="mm", bufs=2, name="pf")
                    for k in range(klo, khi):
                        nc.tensor.matmul(
                            pf[:], outT[k][:, bass.ts(m, P)], wot[:, k],
                            start=(k == 0), stop=(k == KT - 1))
                        yield
                    if khi == KT:
                        osb = ph3.tile([P, 512], F32, tag="osb", bufs=3,
                                       name="osb")
                        nc.vector.tensor_add(osb[:], pf[:], boutb[:, cs])
                        nc.sync.dma_start(out.ap()[bass.ts(m, P), cs], osb[:])
                    else:
                        _final_partial[(n, m)] = pf

                _final_partial = {}

                def run_gen(g):
                    for _ in g:
                        pass

                qt, kt = alloc_qkt()
                run_gen(proj_mms(0, wq0, qt, kt, alt=True))

                for p in range(NP):
                    if p + 1 < NP:
                        wq_n = load_wq(p + 1)
                        if p == NP - 2:
                            wot0 = load_wot(0)
                            wot1 = load_wot(1)
                            wq_last = wq_n
                        qt_n, kt_n = alloc_qkt()
                        fq.add(proj_mms(p + 1, wq_n, qt_n, kt_n,
                                        skip_q_n1=(p + 1 == NP - 1)))
                    else:
                        # pair 7 bank 0 is filler-starved: its deferred q
                        # bank-1 projection (not needed until bank 1) plus
                        # the k<=6 partial final accumulations for seq tiles
                        # 0/1 fill the ACT-bound window. Evict on DVE — ACT
                        # is the pacer here.
                        fq.add(proj_group(NP - 1, wq_last, qt, kt, 0, 1,
                                          act_evict=False))
                        fq.add(emit_final_group(0, 0, wot0, 0, KT - 1))
                        fq.add(emit_final_group(0, 1, wot0, 0, KT - 1))

                    for n in range(NB):
                        cs = bass.ts(n, 512)
                        expAB = ph2.tile([P, SM, 2, 512], BF16, tag="expAB",
                                         bufs=2, name="expAB")
                        poA = psp.tile([P, 512], F32, tag="pv", bufs=2,
                                       name="poA")
                        poB = psp.tile([P, 512], F32, tag="pv", bufs=2,
                                       name="poB")

                        def emit_pv(m):
                            nc.tensor.matmul(
                                poA[:], vaug[m][:, 2 * p, :],
                                expAB[:, m, 0],
                                start=(m == 0), stop=(m == SM - 1))
                            nc.tensor.matmul(
                                poB[:], vaug[m][:, 2 * p + 1, :],
                                expAB[:, m, 1],
                                start=(m == 0), stop=(m == SM - 1))

                        for m in range(SM):
                            # one m-tile per step, both heads in one 2-bank
                            # PSUM tile: the sc ring is then truly double-
                            # buffered (1 alloc/step) so the next step's
                            # scores don't wait on this step's exp
                            psAB = psp.tile([P, 2, 512], F32, tag="sc",
                                            bufs=2, name="psAB")
                            ms = bass.ts(m, P)
                            ia = nc.tensor.matmul(
                                psAB[:, 0], kt[0:D, ms], qt[0:D, cs])
                            ib = nc.tensor.matmul(
                                psAB[:, 1], kt[D:P, ms], qt[D:P, cs])
                            # chain so the two half-array (row-tiled)
                            # matmuls issue back-to-back and overlap
                            add_dep_helper(ib.ins, ia.ins, sync=False,
                                           reason="pair scores order")
                            nc.scalar.activation(
                                expAB[:, m], psAB[:], AF.Exp, scale=SCALE)
                            # the previous step's PV matmuls are ready to run
                            # exactly when emitted (their exp already done)
                            if m > 0:
                                emit_pv(m - 1)
                            fq.pull(3 if p == NP - 1 else 2)
                        emit_pv(SM - 1)
                        fq.pull(2)
                        if p == NP - 1 and n == 1:
                            # the score ring is free after the last exp: park
                            # k<=6 partial accumulations for seq tiles 4/5 in
                            # its four bank-slots so the post-loop drain only
                            # has their k=7 completions plus tiles 6/7
                            for i, (nn, m) in enumerate(
                                    ((0, 4), (1, 4), (0, 5), (1, 5))):
                                if i % 2 == 0:
                                    sct = psp.tile([P, 2, 512], F32, tag="sc",
                                                   bufs=2, name="pfsc")
                                run_gen(emit_final_group(
                                    nn, m, wot0 if nn == 0 else wot1,
                                    0, KT - 1, pf=sct[:, i % 2]))
                        for h, po in ((0, poA), (1, poB)):
                            # po rows 64..127 hold the row-sum replicated 64x
                            # (ones block of V_aug) — no partition broadcast
                            # needed. Stage to SBUF (the fast-reciprocal bit
                            # trick can't read PSUM, and GPSIMD can't touch
                            # PSUM at all).
                            rs64 = ph2.tile([D, 512], F32, tag="rs", bufs=4,
                                            name="rs64")
                            nc.vector.tensor_copy(rs64[:], po[D:2 * D, :])
                            rec = ph2.tile([D, 512], F32, tag="rec", bufs=4,
                                           name="rec")
                            nc.vector.reciprocal_approx_fast(rec[:], rs64[:])
                            nc.vector.tensor_mul(
                                outT[p][h * D:(h + 1) * D, cs],
                                po[0:D, :], rec[:])
                            fq.pull(4)
                        if p == NP - 1 and n == 0:
                            # pair 7 bank 0 just finished: complete the m0/m1
                            # partial groups (k=7 needs pair 7's outT), then
                            # queue full final groups as bank-1 filler
                            fq.finish()
                            for m in range(2):
                                run_gen(emit_final_group(
                                    0, m, wot0, KT - 1, KT,
                                    pf=_final_partial.pop((0, m))))
                            fq.add(emit_final_group(0, 2, wot0))
                            fq.add(emit_final_group(0, 3, wot0))
                            fq.add(emit_final_group(1, 0, wot1))
                            fq.add(emit_final_group(1, 1, wot1))
                            fq.add(emit_final_group(1, 2, wot1))
                            fq.add(emit_final_group(1, 3, wot1))
                    fq.finish()
                    if p + 1 < NP:
                        qt, kt = qt_n, kt_n

                # ---- phase 3: k=7 completions interleaved between the
                # full 6/7 groups so their DVE evictions overlap PE work ----
                completions = [(0, 4), (1, 4), (0, 5), (1, 5)]
                fulls = [(0, 6), (1, 6), (0, 7), (1, 7)]
                for i in range(4):
                    nn, m = completions[i]
                    run_gen(emit_final_group(
                        nn, m, wot0 if nn == 0 else wot1, KT - 1, KT,
                        pf=_final_partial.pop((nn, m))))
                    nn, m = fulls[i]
                    run_gen(emit_final_group(nn, m, wot0 if nn == 0 else wot1))

    nc.finalize()
    return nc


_NC = None


def _get_nc():
    global _NC
    if _NC is None:
        _NC = build_nc()
    return _NC


def _prep_weights(W_qkv, b_qkv):
    # reference column order is (h, d, qkv) with qkv innermost
    W = np.asarray(W_qkv, dtype=np.float32).reshape(E, H, D, 3)
    b = np.asarray(b_qkv, dtype=np.float32).reshape(H, D, 3)
    Wq = W[..., 0].reshape(E, E)
    Wk = W[..., 1].reshape(E, E)
    Wv = W[..., 2].reshape(E, E)
    bq = b[..., 0].reshape(E)
    bk = b[..., 1].reshape(E)
    bv = b[..., 2].reshape(E)
    wqk = np.empty((E, 2 * E), dtype=np.float32)
    bqk = np.empty(2 * E, dtype=np.float32)
    for p in range(NP):
        wqk[:, p * 256:p * 256 + P] = Wq[:, p * P:(p + 1) * P]
        wqk[:, p * 256 + P:(p + 1) * 256] = Wk[:, p * P:(p + 1) * P]
        bqk[p * 256:p * 256 + P] = bq[p * P:(p + 1) * P]
        bqk[p * 256 + P:(p + 1) * 256] = bk[p * P:(p + 1) * P]
    return wqk, np.ascontiguousarray(Wv), bqk, bv


def kernel(x, W_qkv, b_qkv, W_out, b_out, _trace=False, _tmpdir=None):
    bf = ml_dtypes.bfloat16
    x = np.ascontiguousarray(np.asarray(x, dtype=np.float32).astype(bf))
    wqk, wv, bqk, bv = _prep_weights(W_qkv, b_qkv)
    wqk = wqk.astype(bf)
    wv = wv.astype(bf)
    wout = np.ascontiguousarray(
        np.asarray(W_out, dtype=np.float32).astype(bf))
    bout = np.ascontiguousarray(np.asarray(b_out, dtype=np.float32))
    nc = _get_nc()
    in_maps = [
        {"x": np.ascontiguousarray(x[i]), "wqk": wqk, "wv": wv, "bqk": bqk,
         "bv": bv, "wout": wout, "bout": bout}
        for i in range(x.shape[0])
    ]
    res = run_bass_kernel_spmd(
        nc, in_maps, core_ids=list(range(x.shape[0])),
        trace=_trace, tmpdir=_tmpdir)
    outp = np.stack([rr["out"] for rr in res.results], axis=0)
    kernel.last_result = res
    return outp


# revision 37
# speedup vs baseline: 1.1780x; 1.1780x over previous
"""Multi-head attention block on 8 Trainium2 NeuronCores, data-parallel over batch.

Per core (one batch element, S=1024 seq, E=1024 embed, H=16 heads, D=64),
all matmuls in bf16 (inputs cast host-side), fp32 PSUM accumulation:
  xT: x loaded over both HWDGE queues, PE-transposed 8 k-tiles per PSUM bank,
      one wide eviction per m-tile (alternating DVE/ACT)
  V = xT.T @ Wv (seq-major) into V_aug = [V | ones(64)] per head
  qT/kT = W_pair.T @ xT per head-pair, pipelined as PE filler inside the
          previous pair's attention (generator queue, 2 pulls per score step)
  scoresT[s2,s1] = kT.T @ qT, one m-tile per step, both heads row-tiled in
      one 2-bank PSUM tile (true double buffering of the score ring)
  exp on ACT, one call per step (PSUM->SBUF; no max-subtract: logits are
      ~N(0,1.5) so exp cannot overflow fp32)
  PV: psum[128,512] = V_aug.T @ expT -> rows 0..63 unnorm outT, rows 64..127
      the softmax row-sum replicated 64-wide (free PE broadcast), PV matmuls
      interleaved one step behind the scores
  normalize from PSUM: outT = po[0:64] * (1/po[64:128]) (DVE, no broadcast)
  out = outT.T @ W_out + b_out, overlapped with pair 7: m0/m1 partially
      accumulated (k<=6) inside pair 7 bank 0, m2/m3 + wout-bank-1 m0..m3
      as pair-7 bank-1 filler, m4..m7 after

Weights are de-interleaved host-side: reference W_qkv columns are (h, d, qkv)
with qkv innermost; we feed wqk (pair-blocked [q0q1k0k1...]) and wv ((h,d) order).
"""

import ml_dtypes
import numpy as np

import concourse.bacc as bacc
import concourse.bass as bass
import concourse.mybir as mybir
from concourse.bass_utils import run_bass_kernel_spmd
from concourse.masks import make_identity
from concourse.tile import TileContext
from concourse.tile_rust import add_dep_helper

F32 = mybir.dt.float32
BF16 = mybir.dt.bfloat16
AF = mybir.ActivationFunctionType

S = 1024       # sequence length
E = 1024       # embed dim
H = 16         # heads
D = 64         # head dim
P = 128        # partitions
NP = 8         # head pairs
KT = E // P    # contraction tiles (8)
SM = S // P    # seq tiles of 128 (8)
NB = S // 512  # seq banks of 512 (2)
SCALE = 1.0 / np.sqrt(D)


def build_nc():
    nc = bacc.Bacc(trn_type="TRN2", target_bir_lowering=False)
    x = nc.dram_tensor("x", [S, E], BF16, kind="ExternalInput")
    wqk = nc.dram_tensor("wqk", [E, 2 * E], BF16, kind="ExternalInput")
    wv = nc.dram_tensor("wv", [E, E], BF16, kind="ExternalInput")
    bqk = nc.dram_tensor("bqk", [2 * E], F32, kind="ExternalInput")
    bv = nc.dram_tensor("bv", [E], F32, kind="ExternalInput")
    wout = nc.dram_tensor("wout", [E, E], BF16, kind="ExternalInput")
    bout = nc.dram_tensor("bout", [E], F32, kind="ExternalInput")
    out = nc.dram_tensor("out", [S, E], F32, kind="ExternalOutput")

    with TileContext(nc) as tc:
        with (
            tc.tile_pool(name="const", bufs=1) as constp,
            tc.tile_pool(name="persist", bufs=1) as pers,
            tc.tile_pool(name="psum", bufs=1, space="PSUM") as psp,
        ):
            # ---- constants ----
            ones = constp.tile([1, 512], BF16, tag="ones")
            nc.vector.memset(ones[:], 1.0)

            # ---- persistent arrays ----
            # xTall[:, k, s]: feature-major x, written 8 k-tiles per eviction
            xTall = pers.tile([P, KT, S], BF16, tag="xtall", name="xTall")
            # V_aug: 64 value columns + 64 ones columns per head, so the PV
            # matmul replicates the softmax row-sum across 64 PSUM partitions
            # (free partition-broadcast on the PE; M=128 streams no slower
            # than M=66)
            vaug = [pers.tile([P, H, 2 * D], BF16, tag=f"va{m}", name=f"vaug{m}")
                    for m in range(SM)]
            outT = [pers.tile([P, S], BF16, tag=f"ot{p}", name=f"outT{p}")
                    for p in range(NP)]
            wvall = pers.tile([P, 2, KT, 512], BF16, tag="wvall", name="wvall")

            bvb = constp.tile([P, E], F32, tag="bvb")
            boutb = constp.tile([P, E], F32, tag="boutb")
            with (
                tc.tile_pool(name="ph0", bufs=1) as ph0,
                tc.tile_pool(name="ph2", bufs=1) as ph2,
                tc.tile_pool(name="ph3", bufs=1) as ph3,
            ):
                bvr = ph0.tile([1, E], F32, tag="bvr")
                nc.scalar.dma_start(bvr[:], bv.ap()[None, :])
                botr = ph0.tile([1, E], F32, tag="botr")
                nc.scalar.dma_start(botr[:], bout.ap()[None, :])

                # ---- load x split across both HWDGE queues; PE transposes,
                # 8 k-tiles batched per PSUM bank so DVE evicts each m-tile
                # with ONE wide copy instead of 8 tiny ones.
                # (Concurrent XBAR dma-transposes on the two queues corrupt
                # each other — the XBAR is shared — so transpose on PE.)
                identity = constp.tile([P, P], BF16, tag="ident")
                make_identity(nc, identity)
                # batch DMAs: each dma_start costs ~600ns of sequencer
                # issue time, so load x in 2-tile chunks and wv in quarters
                # (4 issues each instead of 8/16)
                xs2 = []
                for c in range(SM // 2):
                    xst = ph0.tile([P, 2, E], BF16, tag="xs", bufs=4, name="xs")
                    eng = nc.sync if c % 2 == 0 else nc.scalar
                    eng.dma_start(
                        xst[:],
                        x.ap()[c * 2 * P:(c + 1) * 2 * P, :].rearrange(
                            "(m p) e -> p m e", p=P))
                    xs2.append(xst)
                for n in range(2):
                    for kh in range(2):
                        eng = nc.sync if kh == 0 else nc.scalar
                        eng.dma_start(
                            wvall[:, n, kh * 4:(kh + 1) * 4, :],
                            wv.ap()[kh * 512:(kh + 1) * 512,
                                    bass.ts(n, 512)].rearrange(
                                "(k p) c -> p k c", p=P))
                for m in range(SM):
                    tp = psp.tile([P, KT, P], BF16, tag="pv", bufs=2, name="tp")
                    for k in range(KT):
                        nc.tensor.transpose(
                            tp[:, k], xs2[m // 2][:, m % 2, bass.ts(k, P)],
                            identity[:])
                    # alternate evictions between DVE and the idle ACT so
                    # neither paces the transpose stream
                    if m % 2 == 0:
                        nc.vector.tensor_copy(xTall[:, :, bass.ts(m, P)], tp[:])
                    else:
                        nc.scalar.copy(xTall[:, :, bass.ts(m, P)], tp[:])

                # per-partition bias columns for q/k (slow strided DMA; late
                # need, keep behind the transposes on the scalar queue)
                bcols = constp.tile([P, 2 * NP], F32, tag="bcols")
                nc.scalar.dma_start(bcols[:], bqk.ap().rearrange("(f p) -> p f", p=P))

                def load_wq(p):
                    w = ph2.tile([P, KT, 256], BF16, tag="wqk", bufs=2,
                                 name="wqk")
                    nc.sync.dma_start(
                        w[:], wqk.ap()[:, bass.ts(p, 256)].rearrange(
                            "(k q) c -> q k c", q=P))
                    return w

                wq0 = load_wq(0)

                # bias broadcasts in bf16 (fp32 matmuls are 4 cycles/row and
                # would head-block the in-order PE queue for ~7us)
                bvr16 = ph0.tile([1, E], BF16, tag="bvr16")
                nc.vector.tensor_copy(bvr16[:], bvr[:])
                botr16 = ph0.tile([1, E], BF16, tag="botr16")
                nc.vector.tensor_copy(botr16[:], botr[:])
                for n in range(2):
                    cs = bass.ts(n, 512)
                    pb = psp.tile([P, 512], F32, tag="mm", bufs=2, name="pb")
                    nc.tensor.matmul(pb[:], ones[0:1, 0:P], bvr16[0:1, cs])
                    nc.vector.tensor_copy(bvb[:, cs], pb[:])
                    pb2 = psp.tile([P, 512], F32, tag="mm", bufs=2, name="pb2")
                    nc.tensor.matmul(pb2[:], ones[0:1, 0:P], botr16[0:1, cs])
                    nc.vector.tensor_copy(boutb[:, cs], pb2[:])

                # ---- phase 1: V = x @ Wv (+bv), into vaug with ones cols.
                # Transposes interleave with the V groups; V/proj0 PSUM
                # alternates between the idle "mm" and "sc" rings for depth 4
                # so the DVE evictions never gate the PE.
                for m in range(SM):
                    nc.vector.memset(vaug[m][:, :, D:2 * D], 1.0)

                def alt_psum(i, name):
                    if i % 2 == 0:
                        return psp.tile([P, 512], F32, tag="mm", bufs=2,
                                        name=name)
                    t = psp.tile([P, 2, 512], F32, tag="sc", bufs=2, name=name)
                    return t[:, 0]

                def emit_v(n, m):
                    pv = alt_psum(m, "pvps")
                    for k in range(KT):
                        nc.tensor.matmul(
                            pv[:], xTall[:, k, bass.ts(m, P)], wvall[:, n, k],
                            start=(k == 0), stop=(k == KT - 1))
                    nc.vector.tensor_add(
                        vaug[m][:, bass.ts(n, 8), 0:D],
                        pv[:].rearrange("p (h d) -> p h d", h=8),
                        bvb[:, bass.ts(n, 512)].rearrange("p (h d) -> p h d", h=8))

                for n in range(2):
                    for m in range(SM):
                        emit_v(n, m)

                # ---- phase 2: attention, software-pipelined over head pairs ----
                def load_wot(n):
                    w = ph3.tile([P, KT, 512], BF16, tag="wo", bufs=2,
                                 name="wot")
                    nc.sync.dma_start(
                        w[:], wout.ap()[:, bass.ts(n, 512)].rearrange(
                            "(k q) c -> q k c", q=P))
                    return w

                def alloc_qkt():
                    qt = ph2.tile([P, S], BF16, tag="qt", bufs=2, name="qt")
                    kt = ph2.tile([P, S], BF16, tag="kt", bufs=2, name="kt")
                    return qt, kt

                def proj_group(p, wq, qt, kt, which, n, alt=False,
                               act_evict=None):
                    """One 8-matmul projection group, yielding per matmul."""
                    ws = slice(which * P, (which + 1) * P)
                    dst = qt if which == 0 else kt
                    bc = bcols[:, 2 * p + which:2 * p + which + 1]
                    cs = bass.ts(n, 512)
                    if alt:
                        ps = alt_psum(2 * which + n, "pproj")
                    else:
                        ps = psp.tile([P, 512], F32, tag="mm", bufs=2,
                                      name="pproj")
                    for k in range(KT):
                        nc.tensor.matmul(
                            ps[:], wq[:, k, ws], xTall[:, k, cs],
                            start=(k == 0), stop=(k == KT - 1))
                        yield
                    if act_evict is None:
                        act_evict = which == 0 and not alt
                    if act_evict:
                        # q eviction on ACT (~1.4us/bank slack) so the
                        # DVE never gates the mm PSUM ring; Identity
                        # supports the per-partition bias column
                        nc.scalar.activation(
                            dst[:, cs], ps[:], AF.Identity, bias=bc)
                    else:
                        nc.vector.tensor_scalar_add(dst[:, cs], ps[:], bc)

                def proj_mms(p, wq, qt, kt, alt=False, skip_q_n1=False):
                    """Generator yielding after each proj matmul. With
                    skip_q_n1, the q bank-1 group is left out (deferred into
                    the pair's own bank-0 window, which only needs q bank 0
                    plus all of k)."""
                    for which in range(2):  # 0 = q, 1 = k
                        for n in range(NB):
                            if skip_q_n1 and which == 0 and n == 1:
                                continue
                            yield from proj_group(p, wq, qt, kt, which, n,
                                                  alt=alt)

                class FQ:
                    def __init__(self):
                        self.q = []

                    def add(self, g):
                        self.q.append(g)

                    def pull(self, n):
                        while n > 0 and self.q:
                            try:
                                next(self.q[0])
                                n -= 1
                            except StopIteration:
                                self.q.pop(0)

                    def finish(self):
                        self.pull(1 << 30)

                fq = FQ()

                def emit_final_group(n, m, wot, klo=0, khi=KT, pf=None):
                    cs = bass.ts(n, 512)
                    if pf is None:
                        pf = psp.tile([P, 512], F32, tag="mm", bufs=2, name="pf")
                    for k in range(klo, khi):
                        nc.tensor.matmul(
                            pf[:], outT[k][:, bass.ts(m, P)], wot[:, k],
                            start=(k == 0), stop=(k == KT - 1))
                        yield
                    if khi == KT:
                        osb = ph3.tile([P, 512], F32, tag="osb", bufs=3,
                                       name="osb")
                        nc.vector.tensor_add(osb[:], pf[:], boutb[:, cs])
                        nc.sync.dma_start(out.ap()[bass.ts(m, P), cs], osb[:])
                    else:
                        _final_partial[(n, m)] = pf

                _final_partial = {}

                def run_gen(g):
                    for _ in g:
                        pass

                qt, kt = alloc_qkt()
                run_gen(proj_mms(0, wq0, qt, kt, alt=True))

                for p in range(NP):
                    if p + 1 < NP:
                        wq_n = load_wq(p + 1)
                        if p == NP - 2:
                            wot0 = load_wot(0)
                            wot1 = load_wot(1)
                            wq_last = wq_n
                        qt_n, kt_n = alloc_qkt()
                        fq.add(proj_mms(p + 1, wq_n, qt_n, kt_n,
                                        skip_q_n1=(p + 1 == NP - 1)))
                    else:
                        # pair 7 bank 0 is filler-starved: its deferred q
                        # bank-1 projection (not needed until bank 1) plus
                        # the k<=6 partial final accumulations for seq tiles
                        # 0/1 fill the ACT-bound window. Evict on DVE — ACT
                        # is the pacer here.
                        fq.add(proj_group(NP - 1, wq_last, qt, kt, 0, 1,
                                          act_evict=False))
                        fq.add(emit_final_group(0, 0, wot0, 0, KT - 1))
                        fq.add(emit_final_group(0, 1, wot0, 0, KT - 1))

                    for n in range(NB):
                        cs = bass.ts(n, 512)
                        expAB = ph2.tile([P, SM, 2, 512], BF16, tag="expAB",
                                         bufs=2, name="expAB")
                        poA = psp.tile([P, 512], F32, tag="pv", bufs=2,
                                       name="poA")
                        poB = psp.tile([P, 512], F32, tag="pv", bufs=2,
                                       name="poB")

                        def emit_pv(m):
                            nc.tensor.matmul(
                                poA[:], vaug[m][:, 2 * p, :],
                                expAB[:, m, 0],
                                start=(m == 0), stop=(m == SM - 1))
                            nc.tensor.matmul(
                                poB[:], vaug[m][:, 2 * p + 1, :],
                                expAB[:, m, 1],
                                start=(m == 0), stop=(m == SM - 1))

                        for m in range(SM):
                            # one m-tile per step, both heads in one 2-bank
                            # PSUM tile: the sc ring is then truly double-
                            # buffered (1 alloc/step) so the next step's
                            # scores don't wait on this step's exp
                            psAB = psp.tile([P, 2, 512], F32, tag="sc",
                                            bufs=2, name="psAB")
                            ms = bass.ts(m, P)
                            ia = nc.tensor.matmul(
                                psAB[:, 0], kt[0:D, ms], qt[0:D, cs])
                            ib = nc.tensor.matmul(
                                psAB[:, 1], kt[D:P, ms], qt[D:P, cs])
                            # chain so the two half-array (row-tiled)
                            # matmuls issue back-to-back and overlap
                            add_dep_helper(ib.ins, ia.ins, sync=False,
                                           reason="pair scores order")
                            nc.scalar.activation(
                                expAB[:, m], psAB[:], AF.Exp, scale=SCALE)
                            # the previous step's PV matmuls are ready to run
                            # exactly when emitted (their exp already done)
                            if m > 0:
                                emit_pv(m - 1)
                            fq.pull(2)
                        emit_pv(SM - 1)
                        fq.pull(2)
                        if p == NP - 1 and n == 1:
                            # the score ring is free after the last exp: park
                            # k<=6 partial accumulations for seq tiles 4/5 in
                            # its four bank-slots so the post-loop drain only
                            # has their k=7 completions plus tiles 6/7
                            for i, (nn, m) in enumerate(
                                    ((0, 4), (1, 4), (0, 5), (1, 5))):
                                if i % 2 == 0:
                                    sct = psp.tile([P, 2, 512], F32, tag="sc",
                                                   bufs=2, name="pfsc")
                                run_gen(emit_final_group(
                                    nn, m, wot0 if nn == 0 else wot1,
                                    0, KT - 1, pf=sct[:, i % 2]))
                        for h, po in ((0, poA), (1, poB)):
                            # po rows 64..127 hold the row-sum replicated 64x
                            # (ones block of V_aug) — no partition broadcast
                            # needed. Stage to SBUF (the fast-reciprocal bit
                            # trick can't read PSUM, and GPSIMD can't touch
                            # PSUM at all).
                            rs64 = ph2.tile([D, 512], F32, tag="rs", bufs=4,
                                            name="rs64")
                            nc.vector.tensor_copy(rs64[:], po[D:2 * D, :])
                            rec = ph2.tile([D, 512], F32, tag="rec", bufs=4,
                                           name="rec")
                            nc.vector.reciprocal_approx_fast(rec[:], rs64[:])
                            nc.vector.tensor_mul(
                                outT[p][h * D:(h + 1) * D, cs],
                                po[0:D, :], rec[:])
                            fq.pull(4)
                        if p == NP - 1 and n == 0:
                            # pair 7 bank 0 just finished: complete the m0/m1
                            # partial groups (k=7 needs pair 7's outT), then
                            # queue full final groups as bank-1 filler
                            fq.finish()
                            for m in range(2):
                                run_gen(emit_final_group(
                                    0, m, wot0, KT - 1, KT,
                                    pf=_final_partial.pop((0, m))))
                            fq.add(emit_final_group(0, 2, wot0))
                            fq.add(emit_final_group(0, 3, wot0))
                            fq.add(emit_final_group(1, 0, wot1))
                            fq.add(emit_final_group(1, 1, wot1))
                            fq.add(emit_final_group(1, 2, wot1))
                            fq.add(emit_final_group(1, 3, wot1))
                    fq.finish()
                    if p + 1 < NP:
                        qt, kt = qt_n, kt_n

                # ---- phase 3: k=7 completions interleaved between the
                # full 6/7 groups so their DVE evictions overlap PE work ----
                completions = [(0, 4), (1, 4), (0, 5), (1, 5)]
                fulls = [(0, 6), (1, 6), (0, 7), (1, 7)]
                for i in range(4):
                    nn, m = completions[i]
                    run_gen(emit_final_group(
                        nn, m, wot0 if nn == 0 else wot1, KT - 1, KT,
                        pf=_final_partial.pop((nn, m))))
                    nn, m = fulls[i]
                    run_gen(emit_final_group(nn, m, wot0 if nn == 0 else wot1))

    nc.finalize()
    return nc


_NC = None


def _get_nc():
    global _NC
    if _NC is None:
        _NC = build_nc()
    return _NC


def _prep_weights(W_qkv, b_qkv):
    # reference column order is (h, d, qkv) with qkv innermost
    W = np.asarray(W_qkv, dtype=np.float32).reshape(E, H, D, 3)
    b = np.asarray(b_qkv, dtype=np.float32).reshape(H, D, 3)
    Wq = W[..., 0].reshape(E, E)
    Wk = W[..., 1].reshape(E, E)
    Wv = W[..., 2].reshape(E, E)
    bq = b[..., 0].reshape(E)
    bk = b[..., 1].reshape(E)
    bv = b[..., 2].reshape(E)
    wqk = np.empty((E, 2 * E), dtype=np.float32)
    bqk = np.empty(2 * E, dtype=np.float32)
    for p in range(NP):
        wqk[:, p * 256:p * 256 + P] = Wq[:, p * P:(p + 1) * P]
        wqk[:, p * 256 + P:(p + 1) * 256] = Wk[:, p * P:(p + 1) * P]
        bqk[p * 256:p * 256 + P] = bq[p * P:(p + 1) * P]
        bqk[p * 256 + P:(p + 1) * 256] = bk[p * P:(p + 1) * P]
    return wqk, np.ascontiguousarray(Wv), bqk, bv


def kernel(x, W_qkv, b_qkv, W_out, b_out, _trace=False, _tmpdir=None):
    bf = ml_dtypes.bfloat16
    x = np.ascontiguousarray(np.asarray(x, dtype=np.float32).astype(bf))
    wqk, wv, bqk, bv = _prep_weights(W_qkv, b_qkv)
    wqk = wqk.astype(bf)
    wv = wv.astype(bf)
    wout = np.ascontiguousarray(
        np.asarray(W_out, dtype=np.float32).astype(bf))
    bout = np.ascontiguousarray(np.asarray(b_out, dtype=np.float32))
    nc = _get_nc()
    in_maps = [
        {"x": np.ascontiguousarray(x[i]), "wqk": wqk, "wv": wv, "bqk": bqk,
         "bv": bv, "wout": wout, "bout": bout}
        for i in range(x.shape[0])
    ]
    res = run_bass_kernel_spmd(
        nc, in_maps, core_ids=list(range(x.shape[0])),
        trace=_trace, tmpdir=_tmpdir)
    outp = np.stack([rr["out"] for rr in res.results], axis=0)
    kernel.last_result = res
    return outp
